# revision 15
# baseline (speedup 1.0000x reference)
"""v9: bf16 single-seed chain build, gpsimd offload, act-table pairing.

vs v8:
- Single Derivative_Erf seed per job; cols 1..JC-1 chained by a step-1 ratio
  r1 = exp(2NFh(v-cJ)) (chain depth 9; W re-fit absorbs rounding bias;
  host-sim end-to-end max-rel err ~6.6e-3 vs the 2e-2 gate).
- Both jobs advanced in ONE vector instruction per step: out cols {m, JC+m}
  as [CH, 2jobs, 2axis] with packed bf16 pairs (DVE 2x mode).
- |d-b| weight and the d-b subtraction run on the idle GPSIMD engine; a
  channel tail of every chain instruction also runs on GPSIMD to offload
  the saturated vector engine.
- ACT emission interleaves table groups per diagram pair (exp ratios ->
  deriv_erf seeds) so vector work starts ~9us in instead of ~35us.
"""

import numpy as np
from contextlib import ExitStack

import concourse.bass as bass
import concourse.bacc as bacc
import concourse.tile as tile
from concourse import mybir

F32 = mybir.dt.float32
BF16 = mybir.dt.bfloat16

RESOLUTION = 50
SIGMA = 0.05
NF = float(np.float32(1.0 / (2.0 * SIGMA**2 + 1e-8)))
SQNF = float(np.float32(np.sqrt(NF)))
XLO, XHI = -0.06, 1.06


def _bf16(x):
    x = np.asarray(x, np.float32)
    u = x.view(np.uint32)
    r = ((u >> 16) & 1).astype(np.uint32)
    return ((u + 0x7FFF + r) & 0xFFFF0000).view(np.float32)


def make_host_constants(Nc=20, njobs=2, ridge=1e-6, nv=16001):
    """Fit W against the bf16-simulated single-seed chain basis."""
    JC = Nc // njobs
    xc = np.linspace(XLO, XHI, Nc)
    h = float(xc[1] - xc[0])
    v = np.linspace(0.0, 1.0, nv).astype(np.float32)
    Phi = np.zeros((nv, Nc), np.float32)
    for j in range(njobs):
        J = np.arange(j * JC, (j + 1) * JC)
        cJ = 0.5 * (xc[J[0]] + xc[J[-1]])
        arg = (np.float32(SQNF) * v + np.float32(-SQNF * xc[J[0]])).astype(np.float64)
        Phi[:, j * JC] = _bf16((2 / np.sqrt(np.pi)) * np.exp(-arg ** 2))
        r1 = _bf16(np.exp(2 * NF * h * (v.astype(np.float64) - cJ)))
        for k in range(1, JC):
            Phi[:, j * JC + k] = _bf16(Phi[:, j * JC + k - 1] * r1)
    x = np.linspace(0, 1, RESOLUTION)
    G = np.exp(-NF * (x[None, :] - v[:, None].astype(np.float64)) ** 2)
    P = Phi.astype(np.float64)
    A = P.T @ P
    W = np.linalg.solve(A + ridge * np.diag(np.diag(A)), P.T @ G)
    centers = np.array([0.5 * (xc[j * JC] + xc[(j + 1) * JC - 1]) for j in range(njobs)])
    return W.astype(np.float32), xc, h, centers


def build_kernel(DG=4, N=65536, Nc=20, njobs=2, CH_G=80, debug=False):
    assert N % 128 == 0
    CH = N // 128
    JC = Nc // njobs
    KB = 4            # channels batched per matmul
    NB = CH // KB
    CH_V = CH - CH_G  # vector handles channels [0, CH_V), gpsimd the rest
    W, xc, h, centers = make_host_constants(Nc, njobs)

    nc = bacc.Bacc("TRN2", target_bir_lowering=False, debug=debug)

    diagrams = nc.declare_dram_parameter("diagrams", [DG, N, 2], F32, isOutput=False)
    wtx_d = nc.declare_dram_parameter("wtx", [Nc, RESOLUTION], F32, isOutput=False)
    wty_d = nc.declare_dram_parameter("wty", [Nc, RESOLUTION], F32, isOutput=False)
    out_d = nc.declare_dram_parameter("out", [DG, RESOLUTION, RESOLUTION], F32, isOutput=True)
    sel_d = nc.declare_dram_parameter("sel", [KB, KB * Nc, Nc], F32, isOutput=False)

    DERF = mybir.ActivationFunctionType.Derivative_Erf
    EXP = mybir.ActivationFunctionType.Exp

    def bcast(ap_obj, n, pos):
        dims = list(ap_obj.ap)
        dims.insert(pos, [0, n])
        return bass.AP(tensor=ap_obj.tensor, offset=ap_obj.offset, ap=dims)

    with ExitStack() as ctx:
        tc = ctx.enter_context(tile.TileContext(nc))
        singles = ctx.enter_context(tc.tile_pool(name="singles", bufs=1))
        raws = ctx.enter_context(tc.tile_pool(name="raws", bufs=4))
        seeds = ctx.enter_context(tc.tile_pool(name="seeds", bufs=4))
        preps = ctx.enter_context(tc.tile_pool(name="preps", bufs=4))
        bigs = ctx.enter_context(tc.tile_pool(name="bigs", bufs=2))
        psums = ctx.enter_context(tc.tile_pool(name="psums", bufs=2, space="PSUM"))
        outs = ctx.enter_context(tc.tile_pool(name="outs", bufs=2))

        wtx_t = singles.tile([Nc, RESOLUTION], F32)
        nc.sync.dma_start(out=wtx_t[:], in_=wtx_d[:])
        wty_t = singles.tile([Nc, RESOLUTION], F32)
        nc.sync.dma_start(out=wty_t[:], in_=wty_d[:])
        sel_t = []
        for q in range(KB):
            st = singles.tile([KB * Nc, Nc], F32, tag=f"sel{q}", name=f"sel{q}")
            nc.sync.dma_start(out=st[:], in_=sel_d[q])
            sel_t.append(st)

        def bias_tile(name, v):
            bt = singles.tile([128, 1], F32, tag=f"bias_{name}", name=f"bias_{name}")
            nc.vector.memset(bt[:], float(v))
            return bt

        seed_bias = {j: bias_tile(f"s{j}", -SQNF * float(xc[j * JC]))
                     for j in range(njobs)}
        r1_bias = {j: bias_tile(f"r{j}", -2 * NF * h * float(centers[j]))
                   for j in range(njobs)}
        zero_bias = bias_tile("z", 0.0)

        raw_t, rawp_t = [], []
        for dg in range(DG):
            raw = raws.tile([128, CH * 2], F32, tag="raw", name=f"raw{dg}")
            dsrc = diagrams[dg].rearrange("(p c) t -> p (c t)", p=128)
            for si, eng in enumerate((nc.sync, nc.scalar, nc.sync, nc.scalar)):
                sl = slice(si * CH * 2 // 4, (si + 1) * CH * 2 // 4)
                eng.dma_start(out=raw[:, sl], in_=dsrc[:, sl])
            raw_t.append(raw)
            rawp_t.append(raw.rearrange("p (c t) -> p c t", t=2))

        # gpsimd: pd = d - b; ACT: pers = |pd| dup'd into the axis pair
        # (Abs lives in every act table -- no table-switch cost)
        ABS = mybir.ActivationFunctionType.Abs
        pers_t = []
        for dg in range(DG):
            rawp = rawp_t[dg]
            pd = preps.tile([128, CH], F32, tag="pd", name=f"pd{dg}")
            nc.gpsimd.tensor_sub(pd[:], rawp[:, :, 1], rawp[:, :, 0])
            pers = preps.tile([128, CH, 2], BF16, tag="pers", name=f"pers{dg}")
            nc.scalar.activation(
                out=pers[:, :, :], in_=bcast(pd[:], 2, 2),
                func=ABS, scale=1.0, bias=zero_bias[:],
            )
            pers_t.append(pers)

        # ACT, paired by diagram couples to bound act-table switches while
        # letting vector work start early: [exp dg01][derf dg01][exp dg23]...
        r1b_t, seed_t = [None] * DG, [None] * DG
        for pair in range(DG // 2):
            dgs = (2 * pair, 2 * pair + 1)
            for dg in dgs:
                r1b = preps.tile([128, CH, njobs, 2], BF16, tag="r1b", name=f"r1b{dg}")
                for j in range(njobs):
                    nc.scalar.activation(
                        out=r1b[:, :, j], in_=rawp_t[dg][:, :, :],
                        func=EXP, scale=float(2 * NF * h), bias=r1_bias[j][:],
                    )
                r1b_t[dg] = r1b
            for dg in dgs:
                sb = seeds.tile([128, CH, njobs, 2], BF16, tag="seed", name=f"seed{dg}")
                for j in range(njobs):
                    nc.scalar.activation(
                        out=sb[:, :, j], in_=rawp_t[dg][:, :, :],
                        func=DERF, scale=SQNF, bias=seed_bias[j][:],
                    )
                seed_t[dg] = sb

        for dg in range(DG):
            rawp = rawp_t[dg]
            pers, r1b, sb = pers_t[dg], r1b_t[dg], seed_t[dg]

            # +1 padding channel (matmul overread target), zeroed
            T = bigs.tile([128, CH + 1, Nc, 2], BF16, tag="T", name=f"T{dg}")
            nc.vector.memset(T[:, CH], 0.0)

            def jcols(t, m, c0, c1):
                """AP over cols {m, JC+m} x axis pair, channels [c0, c1)."""
                v = t[:, c0:c1, m, :]
                return bass.AP(tensor=v.tensor, offset=v.offset,
                               ap=[v.ap[0], v.ap[1], [JC * 2, njobs], v.ap[2]])

            # w-fold: col0 of each job = seed_j * pers  (one instr per engine)
            for eng, c0, c1 in ((nc.vector, 0, CH_V), (nc.gpsimd, CH_V, CH)):
                if c0 == c1:
                    continue
                eng.tensor_mul(
                    jcols(T, 0, c0, c1),
                    sb[:, c0:c1, :, :],
                    bcast(pers[:, c0:c1], njobs, 2),
                )
                for m in range(1, JC):
                    eng.tensor_mul(
                        jcols(T, m, c0, c1),
                        jcols(T, m - 1, c0, c1),
                        r1b[:, c0:c1, :, :],
                    )

            hp = psums.tile([KB * Nc, KB * Nc], F32, tag="H", name=f"H{dg}")
            Tx = T[:, :, :, 0]
            Ty = T[:, :, :, 1]
            for g in range(NB):
                xs = Tx[:, KB * g:KB * (g + 1), :]
                xs1 = bass.AP(tensor=xs.tensor, offset=xs.offset,
                              ap=[xs.ap[0], [2, KB * Nc]])
                ys = Ty[:, KB * g:KB * (g + 1), :]
                ys1 = bass.AP(tensor=ys.tensor, offset=ys.offset,
                              ap=[ys.ap[0], [2, KB * Nc]])
                nc.tensor.matmul(
                    hp[:], xs1, ys1,
                    start=(g == 0), stop=(g == NB - 1),
                )

            # tail: diagonal blocks via selector matmuls (engines need
            # 32-aligned partition bases; selectors avoid offset reads)
            hc = outs.tile([KB * Nc, KB * Nc], F32, tag="hc", name=f"hc{dg}")
            nc.vector.tensor_copy(hc[:], hp[:])
            hps = psums.tile([Nc, Nc], F32, tag="hps", name=f"hps{dg}")
            for q in range(KB):
                nc.tensor.matmul(
                    hps[:], sel_t[q][:], hc[:, Nc * q:Nc * (q + 1)],
                    start=(q == 0), stop=(q == KB - 1),
                )
            hs = outs.tile([Nc, Nc], F32, tag="hs", name=f"hs{dg}")
            nc.vector.tensor_copy(hs[:], hps[:])
            p1 = psums.tile([Nc, RESOLUTION], F32, tag="p1", name=f"p1{dg}")
            nc.tensor.matmul(p1[:], hs[:], wtx_t[:], start=True, stop=True)
            o1 = outs.tile([Nc, RESOLUTION], F32, tag="o1", name=f"o1{dg}")
            nc.vector.tensor_copy(o1[:], p1[:])
            p2 = psums.tile([RESOLUTION, RESOLUTION], F32, tag="p2", name=f"p2{dg}")
            nc.tensor.matmul(p2[:], o1[:], wty_t[:], start=True, stop=True)
            o2 = outs.tile([RESOLUTION, RESOLUTION], F32, tag="o2", name=f"o2{dg}")
            nc.vector.tensor_copy(o2[:], p2[:])
            nc.sync.dma_start(out=out_d[dg], in_=o2[:])

    sel = np.zeros((KB, KB * Nc, Nc), np.float32)
    for q in range(KB):
        sel[q, Nc * q:Nc * (q + 1), :] = np.eye(Nc, dtype=np.float32)
    nc.compile()
    return nc, {"wtx": W.copy(), "wty": W.copy(), "sel": sel}


_CACHE = {}


def _get_built():
    if "k" not in _CACHE:
        _CACHE["k"] = build_kernel(DG=4, N=65536, Nc=20, njobs=2)
    return _CACHE["k"]


def kernel(diagrams):
    """Full-input entry point: diagrams [32, 65536, 2] fp32 -> [32, 50, 50] fp32.

    Shards the batch axis over 8 NeuronCores (4 diagrams each), runs the
    Bass kernel SPMD, gathers per-core outputs.
    """
    from concourse.bass_utils import run_bass_kernel_spmd

    diagrams = np.ascontiguousarray(np.asarray(diagrams, dtype=np.float32))
    B, N, two = diagrams.shape
    assert (B, N, two) == (32, 65536, 2), (B, N, two)
    nc, consts = _get_built()
    in_maps = []
    for core in range(8):
        m = {"diagrams": diagrams[core * 4:(core + 1) * 4]}
        m.update(consts)
        in_maps.append(m)
    res = run_bass_kernel_spmd(nc, in_maps, core_ids=list(range(8)))
    out = np.concatenate([res.results[c]["out"] for c in range(8)], axis=0)
    return out.astype(np.float32)


# revision 17
# speedup vs baseline: 1.3269x; 1.3269x over previous
"""v10: single-seed chains with job-interleaved columns (DVE 2x restored).

Lessons from v8/v9 hardware traces baked in:
- DVE 2x mode needs contiguous 4-element (8-byte) runs; strided 2-elem pairs
  run at 1x. T is laid out [128, CH, JC, 2(job), 2(axis)] so every chain
  step writes/reads contiguous (job, axis) quads; the basis column order
  becomes k' = 2m + j, a permutation absorbed into W's row order.
- GPSIMD TensorTensor on strided bf16 APs is ~3x slower than its cost
  model; chains stay on vector. GPSIMD only zeroes the pad channel and
  hosts two input-DMA queues (keeping them off the scalar engine).
- Single Derivative_Erf seed per job + step-1 exp ratio chain (depth 9);
  ACT tables switch once per diagram pair. W re-fit against the
  bf16-simulated chain absorbs rounding bias (host-sim err ~6.6e-3).
- Tail copies run on the scalar engine (Copy is in every act table).
"""

import numpy as np
from contextlib import ExitStack

import concourse.bass as bass
import concourse.bacc as bacc
import concourse.tile as tile
from concourse import mybir

F32 = mybir.dt.float32
BF16 = mybir.dt.bfloat16

RESOLUTION = 50
SIGMA = 0.05
NF = float(np.float32(1.0 / (2.0 * SIGMA**2 + 1e-8)))
SQNF = float(np.float32(np.sqrt(NF)))
XLO, XHI = -0.06, 1.06


def _bf16(x):
    x = np.asarray(x, np.float32)
    u = x.view(np.uint32)
    r = ((u >> 16) & 1).astype(np.uint32)
    return ((u + 0x7FFF + r) & 0xFFFF0000).view(np.float32)


def make_host_constants(Nc=20, njobs=2, ridge=1e-6, nv=16001):
    """Fit W against the bf16-simulated single-seed chain basis.

    Returns W with rows in kernel column order k' = njobs*m + j.
    """
    JC = Nc // njobs
    xc = np.linspace(XLO, XHI, Nc)
    h = float(xc[1] - xc[0])
    v = np.linspace(0.0, 1.0, nv).astype(np.float32)
    Phi = np.zeros((nv, Nc), np.float32)
    for j in range(njobs):
        J = np.arange(j * JC, (j + 1) * JC)
        cJ = 0.5 * (xc[J[0]] + xc[J[-1]])
        arg = (np.float32(SQNF) * v + np.float32(-SQNF * xc[J[0]])).astype(np.float64)
        Phi[:, j * JC] = _bf16((2 / np.sqrt(np.pi)) * np.exp(-arg ** 2))
        r1 = _bf16(np.exp(2 * NF * h * (v.astype(np.float64) - cJ)))
        for k in range(1, JC):
            Phi[:, j * JC + k] = _bf16(Phi[:, j * JC + k - 1] * r1)
    x = np.linspace(0, 1, RESOLUTION)
    G = np.exp(-NF * (x[None, :] - v[:, None].astype(np.float64)) ** 2)
    P = Phi.astype(np.float64)
    A = P.T @ P
    W = np.linalg.solve(A + ridge * np.diag(np.diag(A)), P.T @ G)
    # permute rows into kernel order k' = njobs*m + j  (from k = j*JC + m)
    perm = np.empty(Nc, np.int64)
    for j in range(njobs):
        for m in range(JC):
            perm[njobs * m + j] = j * JC + m
    Wp = W[perm]
    centers = np.array([0.5 * (xc[j * JC] + xc[(j + 1) * JC - 1]) for j in range(njobs)])
    return Wp.astype(np.float32), xc, h, centers


def build_kernel(DG=4, N=65536, Nc=20, njobs=2, debug=False):
    assert N % 128 == 0
    CH = N // 128
    JC = Nc // njobs
    KB = 4            # channels batched per matmul
    NB = CH // KB
    W, xc, h, centers = make_host_constants(Nc, njobs)

    nc = bacc.Bacc("TRN2", target_bir_lowering=False, debug=debug)

    diagrams = nc.declare_dram_parameter("diagrams", [DG, N, 2], F32, isOutput=False)
    wtx_d = nc.declare_dram_parameter("wtx", [Nc, RESOLUTION], F32, isOutput=False)
    wty_d = nc.declare_dram_parameter("wty", [Nc, RESOLUTION], F32, isOutput=False)
    out_d = nc.declare_dram_parameter("out", [DG, RESOLUTION, RESOLUTION], F32, isOutput=True)
    sel_d = nc.declare_dram_parameter("sel", [KB, KB * Nc, Nc], F32, isOutput=False)

    DERF = mybir.ActivationFunctionType.Derivative_Erf
    EXP = mybir.ActivationFunctionType.Exp
    ABS = mybir.ActivationFunctionType.Abs

    def bcast(ap_obj, n, pos):
        dims = list(ap_obj.ap)
        dims.insert(pos, [0, n])
        return bass.AP(tensor=ap_obj.tensor, offset=ap_obj.offset, ap=dims)

    with ExitStack() as ctx:
        tc = ctx.enter_context(tile.TileContext(nc))
        singles = ctx.enter_context(tc.tile_pool(name="singles", bufs=1))
        raws = ctx.enter_context(tc.tile_pool(name="raws", bufs=4))
        seeds = ctx.enter_context(tc.tile_pool(name="seeds", bufs=4))
        preps = ctx.enter_context(tc.tile_pool(name="preps", bufs=4))
        bigs = ctx.enter_context(tc.tile_pool(name="bigs", bufs=2))
        psums = ctx.enter_context(tc.tile_pool(name="psums", bufs=2, space="PSUM"))
        outs = ctx.enter_context(tc.tile_pool(name="outs", bufs=2))

        wtx_t = singles.tile([Nc, RESOLUTION], F32)
        nc.sync.dma_start(out=wtx_t[:], in_=wtx_d[:])
        wty_t = singles.tile([Nc, RESOLUTION], F32)
        nc.sync.dma_start(out=wty_t[:], in_=wty_d[:])
        sel_t = []
        for q in range(KB):
            st = singles.tile([KB * Nc, Nc], F32, tag=f"sel{q}", name=f"sel{q}")
            nc.sync.dma_start(out=st[:], in_=sel_d[q])
            sel_t.append(st)

        def bias_tile(name, v):
            bt = singles.tile([128, 1], F32, tag=f"bias_{name}", name=f"bias_{name}")
            nc.vector.memset(bt[:], float(v))
            return bt

        seed_bias = {j: bias_tile(f"s{j}", -SQNF * float(xc[j * JC]))
                     for j in range(njobs)}
        r1_bias = {j: bias_tile(f"r{j}", -2 * NF * h * float(centers[j]))
                   for j in range(njobs)}
        zero_bias = bias_tile("z", 0.0)

        raw_t, rawp_t = [], []
        for dg in range(DG):
            raw = raws.tile([128, CH * 2], F32, tag="raw", name=f"raw{dg}")
            dsrc = diagrams[dg].rearrange("(p c) t -> p (c t)", p=128)
            for si, eng in enumerate((nc.sync, nc.gpsimd, nc.sync, nc.gpsimd)):
                sl = slice(si * CH * 2 // 4, (si + 1) * CH * 2 // 4)
                eng.dma_start(out=raw[:, sl], in_=dsrc[:, sl])
            raw_t.append(raw)
            rawp_t.append(raw.rearrange("p (c t) -> p c t", t=2))

        # pd = d - b (vector), pers = |pd| dup'd into the axis pair (ACT Abs,
        # which lives in every act table -- no table-switch cost)
        pers_t = []
        for dg in range(DG):
            rawp = rawp_t[dg]
            pd = preps.tile([128, CH], F32, tag="pd", name=f"pd{dg}")
            nc.vector.tensor_sub(pd[:], rawp[:, :, 1], rawp[:, :, 0])
            pers = preps.tile([128, CH, 2], BF16, tag="pers", name=f"pers{dg}")
            nc.scalar.activation(
                out=pers[:, :, :], in_=bcast(pd[:], 2, 2),
                func=ABS, scale=1.0, bias=zero_bias[:],
            )
            pers_t.append(pers)

        # ACT ratios+seeds, table-grouped per diagram pair
        r1b_t, seed_t = [None] * DG, [None] * DG
        for pair in range(DG // 2):
            dgs = (2 * pair, 2 * pair + 1)
            for dg in dgs:
                r1b = preps.tile([128, CH, njobs, 2], BF16, tag="r1b", name=f"r1b{dg}")
                for j in range(njobs):
                    nc.scalar.activation(
                        out=r1b[:, :, j], in_=rawp_t[dg][:, :, :],
                        func=EXP, scale=float(2 * NF * h), bias=r1_bias[j][:],
                    )
                r1b_t[dg] = r1b
            for dg in dgs:
                sb = seeds.tile([128, CH, njobs, 2], BF16, tag="seed", name=f"seed{dg}")
                for j in range(njobs):
                    nc.scalar.activation(
                        out=sb[:, :, j], in_=rawp_t[dg][:, :, :],
                        func=DERF, scale=SQNF, bias=seed_bias[j][:],
                    )
                seed_t[dg] = sb

        for dg in range(DG):
            pers, r1b, sb = pers_t[dg], r1b_t[dg], seed_t[dg]

            # T[p, ch, m, job, axis]: chain quads contiguous; basis column
            # order k' = 2m + j (W rows permuted to match). +1 pad channel
            # as the matmul overread target.
            T = bigs.tile([128, CH + 1, JC, njobs, 2], BF16, tag="T", name=f"T{dg}")
            nc.gpsimd.memset(T[:, CH], 0.0)

            # w-fold into step 0: T[:,:,0] = seed * pers  [CH, 2, 2] quads
            nc.vector.tensor_mul(
                T[:, :CH, 0], sb[:, :, :, :], bcast(pers[:], njobs, 2),
            )
            for m in range(1, JC):
                nc.vector.tensor_mul(
                    T[:, :CH, m], T[:, :CH, m - 1], r1b[:, :, :, :],
                )

            hp = psums.tile([KB * Nc, KB * Nc], F32, tag="H", name=f"H{dg}")
            for g in range(NB):
                # x/y coeffs of KB channels each form one uniform stride-2
                # progression over the (m, j) sweep
                xs = T[:, KB * g:KB * (g + 1), :, :, 0]
                ys = T[:, KB * g:KB * (g + 1), :, :, 1]
                x0 = bass.AP(tensor=xs.tensor, offset=xs.offset,
                             ap=[xs.ap[0], [2, KB * Nc]])
                y0 = bass.AP(tensor=ys.tensor, offset=ys.offset,
                             ap=[ys.ap[0], [2, KB * Nc]])
                nc.tensor.matmul(
                    hp[:], x0, y0,
                    start=(g == 0), stop=(g == NB - 1),
                )

            # tail: diagonal blocks via selector matmuls; copies on scalar
            hc = outs.tile([KB * Nc, KB * Nc], F32, tag="hc", name=f"hc{dg}")
            nc.scalar.copy(hc[:], hp[:])
            hps = psums.tile([Nc, Nc], F32, tag="hps", name=f"hps{dg}")
            for q in range(KB):
                nc.tensor.matmul(
                    hps[:], sel_t[q][:], hc[:, Nc * q:Nc * (q + 1)],
                    start=(q == 0), stop=(q == KB - 1),
                )
            hs = outs.tile([Nc, Nc], F32, tag="hs", name=f"hs{dg}")
            nc.scalar.copy(hs[:], hps[:])
            p1 = psums.tile([Nc, RESOLUTION], F32, tag="p1", name=f"p1{dg}")
            nc.tensor.matmul(p1[:], hs[:], wtx_t[:], start=True, stop=True)
            o1 = outs.tile([Nc, RESOLUTION], F32, tag="o1", name=f"o1{dg}")
            nc.scalar.copy(o1[:], p1[:])
            p2 = psums.tile([RESOLUTION, RESOLUTION], F32, tag="p2", name=f"p2{dg}")
            nc.tensor.matmul(p2[:], o1[:], wty_t[:], start=True, stop=True)
            o2 = outs.tile([RESOLUTION, RESOLUTION], F32, tag="o2", name=f"o2{dg}")
            nc.scalar.copy(o2[:], p2[:])
            nc.sync.dma_start(out=out_d[dg], in_=o2[:])

    sel = np.zeros((KB, KB * Nc, Nc), np.float32)
    for q in range(KB):
        sel[q, Nc * q:Nc * (q + 1), :] = np.eye(Nc, dtype=np.float32)
    nc.compile()
    return nc, {"wtx": W.copy(), "wty": W.copy(), "sel": sel}


_CACHE = {}


def _get_built():
    if "k" not in _CACHE:
        _CACHE["k"] = build_kernel(DG=4, N=65536, Nc=20, njobs=2)
    return _CACHE["k"]


def kernel(diagrams):
    """Full-input entry point: diagrams [32, 65536, 2] fp32 -> [32, 50, 50] fp32.

    Shards the batch axis over 8 NeuronCores (4 diagrams each), runs the
    Bass kernel SPMD, gathers per-core outputs.
    """
    from concourse.bass_utils import run_bass_kernel_spmd

    diagrams = np.ascontiguousarray(np.asarray(diagrams, dtype=np.float32))
    B, N, two = diagrams.shape
    assert (B, N, two) == (32, 65536, 2), (B, N, two)
    nc, consts = _get_built()
    in_maps = []
    for core in range(8):
        m = {"diagrams": diagrams[core * 4:(core + 1) * 4]}
        m.update(consts)
        in_maps.append(m)
    res = run_bass_kernel_spmd(nc, in_maps, core_ids=list(range(8)))
    out = np.concatenate([res.results[c]["out"] for c in range(8)], axis=0)
    return out.astype(np.float32)
